# revision 8
# baseline (speedup 1.0000x reference)
"""Fused multi-head attention (QKV projection + softmax attention) on 8 TRN2
NeuronCores.

Problem: x [2, 2048, 1024] f32, w_qkv [1024, 3072] f32 ->
         out [2, 16, 2048, 64] f32   (16 heads, head_dim 64)

Sharding: tensor-parallel over heads. Each of the 8 cores owns 2 heads: it
gets the full x (pre-transposed on host to xT [1024, 4096] so no on-device
transposes of x are needed) plus its [1024, 384] slice of w_qkv columns
(q|k|v for its 2 heads) and computes its slice of the output independently.
No collectives. Output leaves the device as O^T [b, h, d, n]; the host
transposes back to [b, h, n, d] while unsharding.

Per-core pipeline:
  1. QKV^T [e, n] = w.T @ xT via PE (f32r), streamed in 8 slabs of 512
     n-columns (DMA of slab s+1 overlaps compute of slab s).
  2. V blocks transposed back to natural [k, d] layout via PE transposes
     (two heads packed per psum tile, row-tiled concurrent), cast to bf16.
  3. Attention, per (batch, 512-query-block):
       S^T[k, q] (heads side by side) = K^T.T @ Q^T in f32r  (row-tiled by
         head -> the two 64-contraction matmuls run concurrently)
       P^T = exp(S^T/8) -> bf16, on the Scalar engine (the kernel's
         critical resource: ~1us per [128, 1024] block, 128 blocks).
         bf16 P is safe: the rounding errors average out across the 2048
         keys of the softmax-weighted sum (~2e-4 contribution).
       O^T[d, q] (+= over k-blocks) = V.T @ P^T in bf16, col-tiled by head
         (fp32r cannot col-tile: ISA allows col_grp != 0xf only for 16/8-bit)
       denom[q] += ones.T @ P^T (M=1 matmuls into psum rows 0/32, col
         groups 0/1, concurrent)
     Epilogue: recip(denom) on DVE, broadcast across partitions with two
     K=64 matmuls against a ones[64,128] stationary (rows 1.. of the rhs
     are zeroed once), multiply on DVE, DMA out.
"""

import numpy as np

import concourse.bass as bass
import concourse.tile as tile
from concourse import bacc, mybir
from concourse.bass_utils import run_bass_kernel_spmd
from concourse.masks import make_identity

F32 = mybir.dt.float32
F32R = mybir.dt.float32r
BF16 = mybir.dt.bfloat16
EXP = mybir.ActivationFunctionType.Exp

B = 2
N = 2048                 # sequence length per batch
NT = B * N               # 4096 columns in xT / qkvT
DIM = 1024
HEADS = 16
HD = 64
NCORES = 8
H_LOC = HEADS // NCORES  # 2 heads per core
E = 3 * H_LOC * HD       # 384 w columns per core
NSLAB = 8
SLAB = NT // NSLAB       # 512 n-columns per projection slab
QB = 512                 # query block
NQB = N // QB            # 4 query blocks per batch
KB = 128                 # key block
NKB = N // KB            # 16 key blocks per batch


def _emit(tc, out_ap, x_ap, w_ap):
    nc = tc.nc
    from contextlib import ExitStack
    ctx = ExitStack()
    with ctx:
        const = ctx.enter_context(tc.tile_pool(name="const", bufs=1))
        xsp = ctx.enter_context(tc.tile_pool(name="xsp", bufs=3))
        qkv = ctx.enter_context(tc.tile_pool(name="qkv", bufs=1))
        vpp = ctx.enter_context(tc.tile_pool(name="vpp", bufs=1))
        ptp = ctx.enter_context(tc.tile_pool(name="ptp", bufs=3))
        smp = ctx.enter_context(tc.tile_pool(name="smp", bufs=4))
        onp = ctx.enter_context(tc.tile_pool(name="onp", bufs=2))

        psS = ctx.enter_context(tc.tile_pool(name="psS", bufs=2, space="PSUM"))
        psO = ctx.enter_context(tc.tile_pool(name="psO", bufs=1, space="PSUM"))
        psD = ctx.enter_context(tc.tile_pool(name="psD", bufs=1, space="PSUM"))
        psQ = ctx.enter_context(tc.tile_pool(name="psQ", bufs=1, space="PSUM"))
        psV = ctx.enter_context(tc.tile_pool(name="psV", bufs=1, space="PSUM"))

        # ---- constants ----
        ident = const.tile([128, 128], F32)
        make_identity(nc, ident)
        ident_r = const.tile([128, 128], F32R)
        nc.vector.tensor_copy(out=ident_r, in_=ident)
        ones_f = const.tile([128, 1], F32)
        nc.vector.memset(ones_f, 1.0)
        ones1 = const.tile([128, 1], BF16)   # denom stationary (bf16: exact 1.0)
        nc.vector.tensor_copy(out=ones1, in_=ones_f)
        onesK_f = const.tile([64, 128], F32)
        nc.vector.memset(onesK_f, 1.0)
        onesK = const.tile([64, 128], F32R)  # denom-broadcast stationary
        nc.vector.tensor_copy(out=onesK, in_=onesK_f)
        # persistent rhs for the broadcast matmuls: only row 0 carries data,
        # rows 1..63 stay zero forever (memset has no f32r encoding -> copy)
        rec_z = const.tile([64, 2 * QB], F32)
        nc.vector.memset(rec_z, 0.0)
        rec_r = qkv.tile([64, 2 * QB], F32R, name="rec_r")
        nc.vector.tensor_copy(out=rec_r, in_=rec_z)
        # preload the exp table set early (overlaps the w DMA)
        warm = const.tile([1, 1], F32)
        nc.scalar.activation(out=warm, in_=ones_f[0:1, :], func=EXP)

        # w [1024, 384] -> [128, 8, 384] (dram already f32r bits)
        w_sb = const.tile([128, 8, E], F32R)
        nc.sync.dma_start(out=w_sb, in_=w_ap.rearrange("(c p) e -> p c e", p=128))

        # ---- projection: QKV^T = w.T @ xT, slab by slab ----
        qT = qkv.tile([128, NT], F32R, name="qT")
        kT = qkv.tile([128, NT], F32R, name="kT")
        vT = qkv.tile([128, NT], F32R, name="vT")
        Vp = vpp.tile([128, B * NKB, 128], BF16, name="Vp")  # natural [k, d]
        x_r = x_ap.rearrange("(c p) n -> p c n", p=128)

        for s in range(NSLAB):
            n0 = s * SLAB
            xs = xsp.tile([128, 8, SLAB], F32R, tag="xs")
            nc.sync.dma_start(out=xs, in_=x_r[:, :, n0:n0 + SLAB])
            for eb in range(3):
                tgt = (qT, kT, vT)[eb]
                ps = psQ.tile([128, SLAB], F32, tag="pq")
                for dc in range(8):
                    nc.tensor.matmul(
                        ps,
                        w_sb[:, dc, eb * 128:(eb + 1) * 128],
                        xs[:, dc, :],
                        start=(dc == 0), stop=(dc == 7))
                nc.vector.tensor_copy(out=tgt[:, n0:n0 + SLAB], in_=ps)
            # V back to natural [k, d] layout (bf16) for this slab's 4 kb:
            # one full 128x128 transpose flips [e, k] -> [k, (h d)]
            for j in range(4):
                kbg = s * 4 + j
                tp = psV.tile([128, 128], F32R, tag="vt")
                nc.tensor.transpose(
                    tp, vT[:, kbg * 128:(kbg + 1) * 128], ident_r)
                nc.vector.tensor_copy(out=Vp[:, kbg, :], in_=tp)

        # ---- attention ----
        for b in range(B):
            for qj in range(NQB):
                q0 = b * N + qj * QB
                OT = psO.tile([128, QB], F32, tag="OT")
                Dn = psD.tile([128, QB], F32, tag="D")
                for kb in range(NKB):
                    kbg = b * NKB + kb
                    k0 = b * N + kb * KB
                    S = psS.tile([128, 2 * QB], F32, tag="S")
                    for h in range(H_LOC):
                        hp = 64 * h
                        nc.tensor.matmul(
                            S[:, h * QB:(h + 1) * QB],
                            kT[hp:hp + 64, k0:k0 + KB],
                            qT[hp:hp + 64, q0:q0 + QB],
                            start=True, stop=True)
                    PT = ptp.tile([128, 2 * QB], BF16, tag="PT")
                    nc.scalar.activation(out=PT, in_=S, func=EXP, scale=0.125)
                    # skip_group_check: the two heads run separate accumulation
                    # groups in disjoint column groups of the same psum bank
                    # (legal on HW via num_active_cols; CoreSim's group check
                    # mis-flattens partition-sliced offsets and false-positives)
                    for h in range(H_LOC):
                        nc.tensor.matmul(
                            OT[h * 64:(h + 1) * 64, :],
                            Vp[:, kbg, h * 64:(h + 1) * 64],
                            PT[:, h * QB:(h + 1) * QB],
                            start=(kb == 0), stop=(kb == NKB - 1),
                            skip_group_check=True)
                    for h in range(H_LOC):
                        nc.tensor.matmul(
                            Dn[h * 32:h * 32 + 1, :],
                            ones1,
                            PT[:, h * QB:(h + 1) * QB],
                            start=(kb == 0), stop=(kb == NKB - 1),
                            skip_group_check=True)
                # epilogue: normalize by the softmax denominator
                rec = smp.tile([1, 2 * QB], F32, tag="rec")
                nc.vector.reciprocal(rec[:, 0:QB], Dn[0:1, :])
                nc.vector.reciprocal(rec[:, QB:2 * QB], Dn[32:33, :])
                nc.vector.tensor_copy(out=rec_r[0:1, :], in_=rec)
                BcS = smp.tile([128, QB], F32, tag="bc")
                Bc0 = psD.tile([128, QB], F32, tag="D")  # reuses denom bank
                nc.tensor.matmul(Bc0, onesK, rec_r[:, 0:QB],
                                 start=True, stop=True)
                nc.vector.tensor_copy(out=BcS[0:64, :], in_=Bc0[0:64, :])
                Bc1 = psD.tile([128, QB], F32, tag="D")
                nc.tensor.matmul(Bc1, onesK, rec_r[:, QB:2 * QB],
                                 start=True, stop=True)
                nc.vector.tensor_copy(out=BcS[64:128, :], in_=Bc1[64:128, :])
                On = onp.tile([128, QB], F32, tag="On")
                nc.vector.tensor_mul(On, OT, BcS)
                nc.sync.dma_start(
                    out=out_ap[b, :, :, qj * QB:(qj + 1) * QB]
                    .rearrange("h d q -> (h d) q"),
                    in_=On)


_CACHED_NC = None


def _build():
    global _CACHED_NC
    if _CACHED_NC is not None:
        return _CACHED_NC
    nc = bacc.Bacc("TRN2", target_bir_lowering=False, debug=False,
                   num_devices=NCORES)
    x = nc.dram_tensor("x", [DIM, NT], F32R, kind="ExternalInput").ap()
    w = nc.dram_tensor("w", [DIM, E], F32R, kind="ExternalInput").ap()
    out = nc.dram_tensor("out", [B, H_LOC, HD, N], F32,
                         kind="ExternalOutput").ap()
    with tile.TileContext(nc) as tc:
        _emit(tc, out, x, w)
    nc.compile()
    _CACHED_NC = nc
    return nc


def _w_slice(w_qkv: np.ndarray, core: int) -> np.ndarray:
    cols = []
    for part in range(3):  # q, k, v column groups of w_qkv
        base = part * HEADS * HD + core * H_LOC * HD
        cols.append(w_qkv[:, base:base + H_LOC * HD])
    return np.ascontiguousarray(np.concatenate(cols, axis=1))


def kernel(x: np.ndarray, w_qkv: np.ndarray, _trace: bool = False):
    nc = _build()
    x = np.asarray(x, dtype=np.float32)
    w_qkv = np.asarray(w_qkv, dtype=np.float32)
    xT = np.ascontiguousarray(x.reshape(NT, DIM).T)  # [1024, 4096]
    in_maps = [{"x": xT, "w": _w_slice(w_qkv, i)} for i in range(NCORES)]
    res = run_bass_kernel_spmd(nc, in_maps, list(range(NCORES)), trace=_trace)
    out = np.empty((B, HEADS, N, HD), np.float32)
    for i in range(NCORES):
        o = res.results[i]["out"]  # [B, H_LOC, HD, N]
        for hl in range(H_LOC):
            out[:, i * H_LOC + hl] = np.swapaxes(o[:, hl], 1, 2)
    if _trace:
        kernel.last_exec_time_ns = res.exec_time_ns
    return out


# revision 12
# speedup vs baseline: 1.1958x; 1.1958x over previous
"""Fused multi-head attention (QKV projection + softmax attention) on 8 TRN2
NeuronCores.

Problem: x [2, 2048, 1024] f32, w_qkv [1024, 3072] f32 ->
         out [2, 16, 2048, 64] f32   (16 heads, head_dim 64)

Sharding: tensor-parallel over heads. Each of the 8 cores owns 2 heads: it
gets the full x (pre-transposed on host to xT [1024, 4096] so no on-device
transposes of x are needed) plus its [1024, 384] slice of w_qkv columns
(q|k|v for its 2 heads) and computes its slice of the output independently.
No collectives. Output leaves the device as O^T [b, h, d, n]; the host
transposes back to [b, h, n, d] while unsharding.

Per-core pipeline:
  1. QKV^T [e, n] = w.T @ xT via PE (f32r), streamed in 8 slabs of 512
     n-columns (DMA of slab s+1 overlaps compute of slab s).
  2. V blocks transposed back to natural [k, d] layout via PE transposes
     (two heads packed per psum tile, row-tiled concurrent), cast to bf16.
  3. Attention, per (batch, 512-query-block):
       S^T[k, q] (heads side by side) = K^T.T @ Q^T in f32r  (row-tiled by
         head -> the two 64-contraction matmuls run concurrently)
       P^T = exp(S^T/8) -> bf16, on the Scalar engine (the kernel's
         critical resource: ~1us per [128, 1024] block, 128 blocks).
         bf16 P is safe: the rounding errors average out across the 2048
         keys of the softmax-weighted sum (~2e-4 contribution).
       O^T[d, q] (+= over k-blocks) = V.T @ P^T in bf16, col-tiled by head
         (fp32r cannot col-tile: ISA allows col_grp != 0xf only for 16/8-bit)
       denom[q] += ones.T @ P^T (M=1 matmuls into psum rows 0/32, col
         groups 0/1, concurrent)
     Epilogue: recip(denom) on DVE, broadcast across partitions with two
     K=64 matmuls against a ones[64,128] stationary (rows 1.. of the rhs
     are zeroed once), multiply on DVE, DMA out.
"""

import numpy as np

import concourse.bass as bass
import concourse.tile as tile
from concourse import bacc, mybir
from concourse.bass_utils import run_bass_kernel_spmd
from concourse.masks import make_identity

F32 = mybir.dt.float32
F32R = mybir.dt.float32r
BF16 = mybir.dt.bfloat16
EXP = mybir.ActivationFunctionType.Exp

B = 2
N = 2048                 # sequence length per batch
NT = B * N               # 4096 columns in xT / qkvT
DIM = 1024
HEADS = 16
HD = 64
NCORES = 8
H_LOC = HEADS // NCORES  # 2 heads per core
E = 3 * H_LOC * HD       # 384 w columns per core
NSLAB = 8
SLAB = NT // NSLAB       # 512 n-columns per projection slab
QB = 512                 # query block
NQB = N // QB            # 4 query blocks per batch
KB = 128                 # key block
NKB = N // KB            # 16 key blocks per batch


def _emit(tc, out_ap, x_ap, w_ap):
    nc = tc.nc
    from contextlib import ExitStack
    ctx = ExitStack()
    with ctx:
        const = ctx.enter_context(tc.tile_pool(name="const", bufs=1))
        xsp = ctx.enter_context(tc.tile_pool(name="xsp", bufs=3))
        qkv = ctx.enter_context(tc.tile_pool(name="qkv", bufs=1))
        vpp = ctx.enter_context(tc.tile_pool(name="vpp", bufs=1))
        ptp = ctx.enter_context(tc.tile_pool(name="ptp", bufs=3))
        smp = ctx.enter_context(tc.tile_pool(name="smp", bufs=4))
        onp = ctx.enter_context(tc.tile_pool(name="onp", bufs=2))

        psS = ctx.enter_context(tc.tile_pool(name="psS", bufs=2, space="PSUM"))
        psO = ctx.enter_context(tc.tile_pool(name="psO", bufs=1, space="PSUM"))
        psD = ctx.enter_context(tc.tile_pool(name="psD", bufs=1, space="PSUM"))
        psQ = ctx.enter_context(tc.tile_pool(name="psQ", bufs=1, space="PSUM"))
        psV = ctx.enter_context(tc.tile_pool(name="psV", bufs=1, space="PSUM"))

        # ---- constants ----
        ident = const.tile([128, 128], F32)
        make_identity(nc, ident)
        ident_r = const.tile([128, 128], F32R)
        nc.vector.tensor_copy(out=ident_r, in_=ident)
        ones_f = const.tile([128, 1], F32)
        nc.vector.memset(ones_f, 1.0)
        ones1 = const.tile([128, 1], BF16)   # denom stationary (bf16: exact 1.0)
        nc.vector.tensor_copy(out=ones1, in_=ones_f)
        onesK = const.tile([64, 128], F32)   # denom-broadcast stationary (fp32)
        nc.vector.memset(onesK, 1.0)
        # persistent rhs for the broadcast matmuls: only row 0 carries data,
        # rows 1..63 stay zero forever
        rec_r = qkv.tile([64, 2 * QB], F32, name="rec_r")
        nc.vector.memset(rec_r, 0.0)
        # preload the exp table set early (overlaps the w DMA)
        warm = const.tile([1, 1], F32)
        nc.scalar.activation(out=warm, in_=ones_f[0:1, :], func=EXP)

        # w [1024, 384] -> [128, 8, 384] (dram already f32r bits)
        w_sb = const.tile([128, 8, E], F32R)
        nc.sync.dma_start(out=w_sb, in_=w_ap.rearrange("(c p) e -> p c e", p=128))

        # ---- projection: QKV^T = w.T @ xT, slab by slab ----
        qT = qkv.tile([128, NT], F32R, name="qT")
        kT = qkv.tile([128, NT], F32R, name="kT")
        vT = qkv.tile([128, NT], F32R, name="vT")
        Vp = vpp.tile([128, B * NKB, 128], BF16, name="Vp")  # natural [k, d]
        x_r = x_ap.rearrange("(c p) n -> p c n", p=128)

        for s in range(NSLAB):
            n0 = s * SLAB
            xs = xsp.tile([128, 8, SLAB], F32R, tag="xs")
            nc.sync.dma_start(out=xs, in_=x_r[:, :, n0:n0 + SLAB])
            for eb in range(3):
                tgt = (qT, kT, vT)[eb]
                ps = psQ.tile([128, SLAB], F32, tag="pq")
                for dc in range(8):
                    nc.tensor.matmul(
                        ps,
                        w_sb[:, dc, eb * 128:(eb + 1) * 128],
                        xs[:, dc, :],
                        start=(dc == 0), stop=(dc == 7))
                nc.vector.tensor_copy(out=tgt[:, n0:n0 + SLAB], in_=ps)
            # V back to natural [k, d] layout (bf16) for this slab's 4 kb:
            # one full 128x128 transpose flips [e, k] -> [k, (h d)]
            for j in range(4):
                kbg = s * 4 + j
                tp = psV.tile([128, 128], F32R, tag="vt")
                nc.tensor.transpose(
                    tp, vT[:, kbg * 128:(kbg + 1) * 128], ident_r)
                nc.vector.tensor_copy(out=Vp[:, kbg, :], in_=tp)

        # ---- attention ----
        for b in range(B):
            for qj in range(NQB):
                q0 = b * N + qj * QB
                OT = psO.tile([128, QB], F32, tag="OT")
                Dn = psD.tile([128, QB], F32, tag="D")
                for kb in range(NKB):
                    kbg = b * NKB + kb
                    k0 = b * N + kb * KB
                    S = psS.tile([128, 2 * QB], F32, tag="S")
                    for h in range(H_LOC):
                        hp = 64 * h
                        nc.tensor.matmul(
                            S[:, h * QB:(h + 1) * QB],
                            kT[hp:hp + 64, k0:k0 + KB],
                            qT[hp:hp + 64, q0:q0 + QB],
                            start=True, stop=True)
                    PT = ptp.tile([128, 2 * QB], BF16, tag="PT")
                    nc.scalar.activation(out=PT, in_=S, func=EXP, scale=0.125)
                    # skip_group_check: the two heads run separate accumulation
                    # groups in disjoint column groups of the same psum bank
                    # (legal on HW via num_active_cols; CoreSim's group check
                    # mis-flattens partition-sliced offsets and false-positives)
                    for h in range(H_LOC):
                        nc.tensor.matmul(
                            OT[h * 64:(h + 1) * 64, :],
                            Vp[:, kbg, h * 64:(h + 1) * 64],
                            PT[:, h * QB:(h + 1) * QB],
                            start=(kb == 0), stop=(kb == NKB - 1),
                            skip_group_check=True)
                    for h in range(H_LOC):
                        nc.tensor.matmul(
                            Dn[h * 32:h * 32 + 1, :],
                            ones1,
                            PT[:, h * QB:(h + 1) * QB],
                            start=(kb == 0), stop=(kb == NKB - 1),
                            skip_group_check=True)
                # epilogue: normalize by the softmax denominator. Everything
                # here is decoupled from the hot S/PT/OT/Dn rotation as fast
                # as possible so the exp stream never stalls: OT and Dn are
                # drained immediately; the broadcast matmuls go to the psV
                # pool (idle once projection is done).
                Osb = onp.tile([128, QB], F32, tag="Os")
                nc.vector.tensor_copy(out=Osb, in_=OT)          # frees psO
                # NB: reciprocal_approx_fast misreads PSUM at partition base
                # 32 on HW (custom-DVE uop quirk) -> drain Dn via plain
                # copies first, then one approx recip straight into rec_r.
                dnb = smp.tile([1, 2 * QB], F32, tag="dn")
                nc.vector.tensor_copy(out=dnb[:, 0:QB], in_=Dn[0:1, :])
                nc.vector.tensor_copy(out=dnb[:, QB:2 * QB], in_=Dn[32:33, :])
                nc.vector.reciprocal_approx_fast(out=rec_r[0:1, :], in_=dnb)
                On = onp.tile([128, QB], F32, tag="On")
                for h in range(H_LOC):
                    Bc = psQ.tile([128, QB], F32, tag="pq", name=f"bc{h}")
                    nc.tensor.matmul(Bc, onesK,
                                     rec_r[:, h * QB:(h + 1) * QB],
                                     start=True, stop=True)
                    nc.vector.tensor_mul(
                        On[h * 64:(h + 1) * 64, :],
                        Osb[h * 64:(h + 1) * 64, :],
                        Bc[h * 64:(h + 1) * 64, :])
                nc.sync.dma_start(
                    out=out_ap[b, :, :, qj * QB:(qj + 1) * QB]
                    .rearrange("h d q -> (h d) q"),
                    in_=On)


_CACHED_NC = None


def _build():
    global _CACHED_NC
    if _CACHED_NC is not None:
        return _CACHED_NC
    nc = bacc.Bacc("TRN2", target_bir_lowering=False, debug=False,
                   num_devices=NCORES)
    x = nc.dram_tensor("x", [DIM, NT], F32R, kind="ExternalInput").ap()
    w = nc.dram_tensor("w", [DIM, E], F32R, kind="ExternalInput").ap()
    out = nc.dram_tensor("out", [B, H_LOC, HD, N], F32,
                         kind="ExternalOutput").ap()
    with tile.TileContext(nc) as tc:
        _emit(tc, out, x, w)
    nc.compile()
    _CACHED_NC = nc
    return nc


def _w_slice(w_qkv: np.ndarray, core: int) -> np.ndarray:
    cols = []
    for part in range(3):  # q, k, v column groups of w_qkv
        base = part * HEADS * HD + core * H_LOC * HD
        cols.append(w_qkv[:, base:base + H_LOC * HD])
    return np.ascontiguousarray(np.concatenate(cols, axis=1))


def kernel(x: np.ndarray, w_qkv: np.ndarray, _trace: bool = False):
    nc = _build()
    x = np.asarray(x, dtype=np.float32)
    w_qkv = np.asarray(w_qkv, dtype=np.float32)
    xT = np.ascontiguousarray(x.reshape(NT, DIM).T)  # [1024, 4096]
    in_maps = [{"x": xT, "w": _w_slice(w_qkv, i)} for i in range(NCORES)]
    res = run_bass_kernel_spmd(nc, in_maps, list(range(NCORES)), trace=_trace)
    out = np.empty((B, HEADS, N, HD), np.float32)
    for i in range(NCORES):
        o = res.results[i]["out"]  # [B, H_LOC, HD, N]
        for hl in range(H_LOC):
            out[:, i * H_LOC + hl] = np.swapaxes(o[:, hl], 1, 2)
    if _trace:
        kernel.last_exec_time_ns = res.exec_time_ns
    return out


# revision 16
# speedup vs baseline: 1.2110x; 1.0127x over previous
"""Fused multi-head attention (QKV projection + softmax attention) on 8 TRN2
NeuronCores.

Problem: x [2, 2048, 1024] f32, w_qkv [1024, 3072] f32 ->
         out [2, 16, 2048, 64] f32   (16 heads, head_dim 64)

Sharding: tensor-parallel over heads. Each of the 8 cores owns 2 heads: it
gets the full x (pre-transposed on host to xT [1024, 4096] so no on-device
transposes of x are needed) plus its [1024, 384] slice of w_qkv columns
(q|k|v for its 2 heads) and computes its slice of the output independently.
No collectives. Output leaves the device as O^T [b, h, d, n]; the host
transposes back to [b, h, n, d] while unsharding.

Per-core pipeline:
  1. QKV^T [e, n] = w.T @ xT via PE (f32r), streamed in 8 slabs of 512
     n-columns (DMA of slab s+1 overlaps compute of slab s).
  2. V blocks transposed back to natural [k, d] layout via PE transposes
     (two heads packed per psum tile, row-tiled concurrent), cast to bf16.
  3. Attention, per (batch, 512-query-block):
       S^T[k, q] (heads side by side) = K^T.T @ Q^T in f32r  (row-tiled by
         head -> the two 64-contraction matmuls run concurrently)
       P^T = exp(S^T/8) -> bf16, on the Scalar engine (the kernel's
         critical resource: ~1us per [128, 1024] block, 128 blocks).
         bf16 P is safe: the rounding errors average out across the 2048
         keys of the softmax-weighted sum (~2e-4 contribution).
       O^T[d, q] (+= over k-blocks) = V.T @ P^T in bf16, col-tiled by head
         (fp32r cannot col-tile: ISA allows col_grp != 0xf only for 16/8-bit)
       denom[q] += ones.T @ P^T (M=1 matmuls into psum rows 0/32, col
         groups 0/1, concurrent)
     Epilogue: recip(denom) on DVE, broadcast across partitions with two
     K=64 matmuls against a ones[64,128] stationary (rows 1.. of the rhs
     are zeroed once), multiply on DVE, DMA out.
"""

import numpy as np

import concourse.bass as bass
import concourse.tile as tile
from concourse import bacc, mybir
from concourse.bass_utils import run_bass_kernel_spmd
from concourse.masks import make_identity

F32 = mybir.dt.float32
F32R = mybir.dt.float32r
BF16 = mybir.dt.bfloat16
EXP = mybir.ActivationFunctionType.Exp

B = 2
N = 2048                 # sequence length per batch
NT = B * N               # 4096 columns in xT / qkvT
DIM = 1024
HEADS = 16
HD = 64
NCORES = 8
H_LOC = HEADS // NCORES  # 2 heads per core
E = 3 * H_LOC * HD       # 384 w columns per core
NSLAB = 8
SLAB = NT // NSLAB       # 512 n-columns per projection slab
QB = 512                 # query block
NQB = N // QB            # 4 query blocks per batch
KB = 128                 # key block
NKB = N // KB            # 16 key blocks per batch


def _emit(tc, out_ap, x_ap, w_ap):
    nc = tc.nc
    from contextlib import ExitStack
    ctx = ExitStack()
    with ctx:
        const = ctx.enter_context(tc.tile_pool(name="const", bufs=1))
        xsp = ctx.enter_context(tc.tile_pool(name="xsp", bufs=3))
        qkv = ctx.enter_context(tc.tile_pool(name="qkv", bufs=1))
        vpp = ctx.enter_context(tc.tile_pool(name="vpp", bufs=1))
        ptp = ctx.enter_context(tc.tile_pool(name="ptp", bufs=3))
        smp = ctx.enter_context(tc.tile_pool(name="smp", bufs=4))
        onp = ctx.enter_context(tc.tile_pool(name="onp", bufs=2))

        psS = ctx.enter_context(tc.tile_pool(name="psS", bufs=2, space="PSUM"))
        psO = ctx.enter_context(tc.tile_pool(name="psO", bufs=1, space="PSUM"))
        psD = ctx.enter_context(tc.tile_pool(name="psD", bufs=1, space="PSUM"))
        psQ = ctx.enter_context(tc.tile_pool(name="psQ", bufs=1, space="PSUM"))
        psV = ctx.enter_context(tc.tile_pool(name="psV", bufs=1, space="PSUM"))

        # ---- constants ----
        ident = const.tile([128, 128], F32)
        make_identity(nc, ident)
        ident_r = const.tile([128, 128], F32R)
        nc.vector.tensor_copy(out=ident_r, in_=ident)
        ones_f = const.tile([128, 1], F32)
        nc.vector.memset(ones_f, 1.0)
        ones1 = const.tile([128, 1], BF16)   # denom stationary (bf16: exact 1.0)
        nc.vector.tensor_copy(out=ones1, in_=ones_f)
        onesK = const.tile([64, 128], F32)   # denom-broadcast stationary (fp32)
        nc.vector.memset(onesK, 1.0)
        # persistent rhs for the broadcast matmuls: only row 0 carries data,
        # rows 1..63 stay zero forever
        rec_r = qkv.tile([64, 2 * QB], F32, name="rec_r")
        nc.vector.memset(rec_r, 0.0)
        # preload the exp table set early (overlaps the w DMA)
        warm = const.tile([1, 1], F32)
        nc.scalar.activation(out=warm, in_=ones_f[0:1, :], func=EXP)

        # w [1024, 384] -> [128, 8, 384] (dram already f32r bits)
        w_sb = const.tile([128, 8, E], F32R)
        nc.sync.dma_start(out=w_sb, in_=w_ap.rearrange("(c p) e -> p c e", p=128))

        # ---- projection: QKV^T = w.T @ xT, slab by slab ----
        qT = qkv.tile([128, NT], F32R, name="qT")
        kT = qkv.tile([128, NT], F32R, name="kT")
        vT = qkv.tile([128, NT], F32R, name="vT")
        Vp = vpp.tile([128, B * NKB, 128], BF16, name="Vp")  # natural [k, d]

        for s in range(NSLAB):
            n0 = s * SLAB
            xs = xsp.tile([128, 8, SLAB], F32R, tag="xs")
            # x arrives pre-slabbed [s, p, c, j]: each partition's 16 KB is
            # contiguous in dram -> full-rate DMA (the [1024, 4096] layout
            # was descriptor-limited at ~2 KB per line and paced the whole
            # ramp, throttling the PE clock)
            nc.sync.dma_start(out=xs, in_=x_ap[s])
            for eb in range(3):
                tgt = (qT, kT, vT)[eb]
                ps = psQ.tile([128, SLAB], F32, tag="pq")
                for dc in range(8):
                    nc.tensor.matmul(
                        ps,
                        w_sb[:, dc, eb * 128:(eb + 1) * 128],
                        xs[:, dc, :],
                        start=(dc == 0), stop=(dc == 7))
                nc.vector.tensor_copy(out=tgt[:, n0:n0 + SLAB], in_=ps)
            # V back to natural [k, d] layout (bf16) for this slab's 4 kb:
            # one full 128x128 transpose flips [e, k] -> [k, (h d)]
            for j in range(4):
                kbg = s * 4 + j
                tp = psV.tile([128, 128], F32R, tag="vt")
                nc.tensor.transpose(
                    tp, vT[:, kbg * 128:(kbg + 1) * 128], ident_r)
                nc.vector.tensor_copy(out=Vp[:, kbg, :], in_=tp)

        # ---- attention ----
        for b in range(B):
            for qj in range(NQB):
                q0 = b * N + qj * QB
                OT = psO.tile([128, QB], F32, tag="OT")
                Dn = psD.tile([128, QB], F32, tag="D")
                for kb in range(NKB):
                    kbg = b * NKB + kb
                    k0 = b * N + kb * KB
                    S = psS.tile([128, 2 * QB], F32, tag="S")
                    for h in range(H_LOC):
                        hp = 64 * h
                        nc.tensor.matmul(
                            S[:, h * QB:(h + 1) * QB],
                            kT[hp:hp + 64, k0:k0 + KB],
                            qT[hp:hp + 64, q0:q0 + QB],
                            start=True, stop=True)
                    PT = ptp.tile([128, 2 * QB], BF16, tag="PT")
                    nc.scalar.activation(out=PT, in_=S, func=EXP, scale=0.125)
                    # skip_group_check: the two heads run separate accumulation
                    # groups in disjoint column groups of the same psum bank
                    # (legal on HW via num_active_cols; CoreSim's group check
                    # mis-flattens partition-sliced offsets and false-positives)
                    for h in range(H_LOC):
                        nc.tensor.matmul(
                            OT[h * 64:(h + 1) * 64, :],
                            Vp[:, kbg, h * 64:(h + 1) * 64],
                            PT[:, h * QB:(h + 1) * QB],
                            start=(kb == 0), stop=(kb == NKB - 1),
                            skip_group_check=True)
                    for h in range(H_LOC):
                        nc.tensor.matmul(
                            Dn[h * 32:h * 32 + 1, :],
                            ones1,
                            PT[:, h * QB:(h + 1) * QB],
                            start=(kb == 0), stop=(kb == NKB - 1),
                            skip_group_check=True)
                # epilogue: normalize by the softmax denominator. Everything
                # here is decoupled from the hot S/PT/OT/Dn rotation as fast
                # as possible so the exp stream never stalls: OT and Dn are
                # drained immediately; the broadcast matmuls go to the psV
                # pool (idle once projection is done).
                Osb = onp.tile([128, QB], F32, tag="Os")
                nc.vector.tensor_copy(out=Osb, in_=OT)          # frees psO
                # NB: reciprocal_approx_fast misreads PSUM at partition base
                # 32 on HW (custom-DVE uop quirk) -> drain Dn via plain
                # copies first, then one approx recip straight into rec_r.
                dnb = smp.tile([1, 2 * QB], F32, tag="dn")
                nc.vector.tensor_copy(out=dnb[:, 0:QB], in_=Dn[0:1, :])
                nc.vector.tensor_copy(out=dnb[:, QB:2 * QB], in_=Dn[32:33, :])
                nc.vector.reciprocal_approx_fast(out=rec_r[0:1, :], in_=dnb)
                On = onp.tile([128, QB], F32, tag="On")
                for h in range(H_LOC):
                    Bc = psQ.tile([128, QB], F32, tag="pq", name=f"bc{h}")
                    nc.tensor.matmul(Bc, onesK,
                                     rec_r[:, h * QB:(h + 1) * QB],
                                     start=True, stop=True)
                    nc.vector.tensor_mul(
                        On[h * 64:(h + 1) * 64, :],
                        Osb[h * 64:(h + 1) * 64, :],
                        Bc[h * 64:(h + 1) * 64, :])
                # scalar-queue HWDGE: keeps output stores off the sync queue
                # so they never delay a slab load mid-ramp
                nc.scalar.dma_start(
                    out=out_ap[b, :, :, qj * QB:(qj + 1) * QB]
                    .rearrange("h d q -> (h d) q"),
                    in_=On)


_CACHED_NC = None


def _build():
    global _CACHED_NC
    if _CACHED_NC is not None:
        return _CACHED_NC
    nc = bacc.Bacc("TRN2", target_bir_lowering=False, debug=False,
                   num_devices=NCORES)
    x = nc.dram_tensor("x", [NSLAB, 128, 8, SLAB], F32R,
                       kind="ExternalInput").ap()
    w = nc.dram_tensor("w", [DIM, E], F32R, kind="ExternalInput").ap()
    out = nc.dram_tensor("out", [B, H_LOC, HD, N], F32,
                         kind="ExternalOutput").ap()
    with tile.TileContext(nc) as tc:
        _emit(tc, out, x, w)
    nc.compile()
    _CACHED_NC = nc
    return nc


def _w_slice(w_qkv: np.ndarray, core: int) -> np.ndarray:
    cols = []
    for part in range(3):  # q, k, v column groups of w_qkv
        base = part * HEADS * HD + core * H_LOC * HD
        cols.append(w_qkv[:, base:base + H_LOC * HD])
    return np.ascontiguousarray(np.concatenate(cols, axis=1))


def kernel(x: np.ndarray, w_qkv: np.ndarray, _trace: bool = False):
    nc = _build()
    x = np.asarray(x, dtype=np.float32)
    w_qkv = np.asarray(w_qkv, dtype=np.float32)
    # pre-slabbed transpose: xup[s, p, c, j] = x[s*512+j, c*128+p]
    xup = np.ascontiguousarray(
        x.reshape(NSLAB, SLAB, 8, 128).transpose(0, 3, 2, 1))
    in_maps = [{"x": xup, "w": _w_slice(w_qkv, i)} for i in range(NCORES)]
    res = run_bass_kernel_spmd(nc, in_maps, list(range(NCORES)), trace=_trace)
    out = np.empty((B, HEADS, N, HD), np.float32)
    for i in range(NCORES):
        o = res.results[i]["out"]  # [B, H_LOC, HD, N]
        for hl in range(H_LOC):
            out[:, i * H_LOC + hl] = np.swapaxes(o[:, hl], 1, 2)
    if _trace:
        kernel.last_exec_time_ns = res.exec_time_ns
    return out


# revision 20
# speedup vs baseline: 1.5849x; 1.3088x over previous
"""Fused multi-head attention (QKV projection + softmax attention) on 8 TRN2
NeuronCores.

Problem: x [2, 2048, 1024] f32, w_qkv [1024, 3072] f32 ->
         out [2, 16, 2048, 64] f32   (16 heads, head_dim 64)

Sharding: tensor-parallel over heads. Each of the 8 cores owns 2 heads: it
gets the full x (pre-transposed AND pre-slabbed on host so every DMA line is
16 KB contiguous) plus its [1024, 384] slice of w_qkv columns and computes
its slice of the output independently. No collectives. Output leaves the
device as O^T per head [b, h, d, n]; the host transposes back to
[b, h, n, d] while unsharding.

Per-core pipeline (everything f32r on the PE = full-rate fp32):
  1. QKV^T [e, n] = w.T @ xT, streamed in 8 slabs of 512 n-columns; psQ is
     double-buffered so the 8-matmul accumulation of the next e-block never
     waits on the DVE drain of the previous one (single-buffering here paced
     the whole ramp at 9.5us/slab, starved the scalar engine, and let the
     PE clock-gate down to 1.2 GHz for the rest of the kernel).
  2. V blocks flipped to natural [k, d] via one full 128x128 PE transpose
     per key-block, stored as Vp = [V_h0 | ones | V_h1 | ones] (65-column
     stationary per head: the ones column accumulates the softmax
     denominator inside the same PV matmul).
  3. Attention, per (batch, 512-query block), pipelined over 16 key-blocks:
       S^T[k, q] = K^T.T @ Q^T   (2 heads row-tiled -> concurrent)
       P^T = exp(S^T/8)           (Scalar engine, the critical resource:
                                   ~1.1us per [128, 1024] block, 128 blocks)
       O^T/denom (+= over kb) = [V_h | 1].T @ P^T_h   (M=65, 2 serial MMs)
     Epilogue (decoupled from the hot psum rotation): denom rows -> SBUF,
     one reciprocal_approx_fast, partition-broadcast via K=64 fp32 matmuls
     against a ones stationary, DVE multiply, per-head DMA out on the
     scalar queue.
"""

import numpy as np

import concourse.bass as bass
import concourse.tile as tile
from concourse import bacc, mybir
from concourse.bass_utils import run_bass_kernel_spmd
from concourse.masks import make_identity

F32 = mybir.dt.float32
F32R = mybir.dt.float32r
EXP = mybir.ActivationFunctionType.Exp

B = 2
N = 2048                 # sequence length per batch
NT = B * N               # 4096 columns in xT / qkvT
DIM = 1024
HEADS = 16
HD = 64
NCORES = 8
H_LOC = HEADS // NCORES  # 2 heads per core
E = 3 * H_LOC * HD       # 384 w columns per core
NSLAB = 8
SLAB = NT // NSLAB       # 512 n-columns per projection slab
QB = 512                 # query block
NQB = N // QB            # 4 query blocks per batch
KB = 128                 # key block
NKB = N // KB            # 16 key blocks per batch


def _emit(tc, out_ap, x_ap, w_ap):
    nc = tc.nc
    from contextlib import ExitStack
    ctx = ExitStack()
    with ctx:
        const = ctx.enter_context(tc.tile_pool(name="const", bufs=1))
        xsp = ctx.enter_context(tc.tile_pool(name="xsp", bufs=3))
        qkv = ctx.enter_context(tc.tile_pool(name="qkv", bufs=1))
        vpp = ctx.enter_context(tc.tile_pool(name="vpp", bufs=1))
        ptp = ctx.enter_context(tc.tile_pool(name="ptp", bufs=3))
        smp = ctx.enter_context(tc.tile_pool(name="smp", bufs=4))
        onp = ctx.enter_context(tc.tile_pool(name="onp", bufs=2))

        psS = ctx.enter_context(tc.tile_pool(name="psS", bufs=2, space="PSUM"))
        psO = ctx.enter_context(tc.tile_pool(name="psO", bufs=2, space="PSUM"))
        psQ = ctx.enter_context(tc.tile_pool(name="psQ", bufs=2, space="PSUM"))

        # ---- constants ----
        ident = const.tile([128, 128], F32)
        make_identity(nc, ident)
        ident_r = const.tile([128, 128], F32R)
        nc.vector.tensor_copy(out=ident_r, in_=ident)
        onesK = const.tile([64, 128], F32)   # denom-broadcast stationary
        nc.vector.memset(onesK, 1.0)
        # persistent rhs for the broadcast matmuls: only row 0 carries data,
        # rows 1..63 stay zero forever
        rec_r = qkv.tile([64, 2 * QB], F32, name="rec_r")
        nc.vector.memset(rec_r, 0.0)
        # preload the exp table set early (overlaps the w DMA)
        warm = const.tile([1, 1], F32)
        nc.scalar.activation(out=warm, in_=onesK[0:1, 0:1], func=EXP)

        # w [1024, 384] -> [128, 8, 384] (dram already f32r bits)
        w_sb = const.tile([128, 8, E], F32R)
        nc.sync.dma_start(out=w_sb, in_=w_ap.rearrange("(c p) e -> p c e", p=128))

        # ---- projection: QKV^T = w.T @ xT, slab by slab ----
        qT = qkv.tile([128, NT], F32R, name="qT")
        kT = qkv.tile([128, NT], F32R, name="kT")
        vT = qkv.tile([128, NT], F32, name="vT")
        # Vp[:, kb] = [V_h0 | ones | V_h1 | ones], natural [k, d] layout
        Vp = vpp.tile([128, B * NKB, 130], F32R, name="Vp")
        Vp_td = Vp.rearrange("p k (t u) -> p k t u", t=2)
        ones32 = const.tile([128, B * NKB], F32)
        nc.vector.memset(ones32, 1.0)
        nc.vector.tensor_copy(out=Vp[:, :, 64:65].squeeze(2), in_=ones32)
        nc.vector.tensor_copy(out=Vp[:, :, 129:130].squeeze(2), in_=ones32)

        for s in range(NSLAB):
            n0 = s * SLAB
            xs = xsp.tile([128, 8, SLAB], F32R, tag="xs")
            nc.sync.dma_start(out=xs, in_=x_ap[s])
            for eb in range(3):
                dt = F32 if eb == 2 else F32R
                tgt = (qT, kT, vT)[eb]
                ps = psQ.tile([128, SLAB], F32, tag="pq", name=f"pq{eb}")
                for dc in range(8):
                    nc.tensor.matmul(
                        ps,
                        w_sb[:, dc, eb * 128:(eb + 1) * 128],
                        xs[:, dc, :],
                        start=(dc == 0), stop=(dc == 7))
                nc.vector.tensor_copy(out=tgt[:, n0:n0 + SLAB], in_=ps)
            # V to natural [k, d] (one full 128x128 transpose per key-block)
            for j in range(4):
                kbg = s * 4 + j
                tp = psQ.tile([128, SLAB], F32, tag="pq", name="vt")
                nc.tensor.transpose(
                    tp[:, 0:128], vT[:, kbg * 128:(kbg + 1) * 128], ident)
                nc.vector.tensor_copy(
                    out=Vp_td[:, kbg, :, 0:64],
                    in_=tp[:, 0:128].rearrange("p (t u) -> p t u", t=2))

        # ---- attention ----
        for b in range(B):
            for qj in range(NQB):
                q0 = b * N + qj * QB
                OT = [psO.tile([65, QB], F32, tag="OT", name=f"OT{h}")
                      for h in range(H_LOC)]
                for kb in range(NKB):
                    kbg = b * NKB + kb
                    k0 = b * N + kb * KB
                    S = psS.tile([128, 2 * QB], F32, tag="S")
                    for h in range(H_LOC):
                        hp = 64 * h
                        nc.tensor.matmul(
                            S[:, h * QB:(h + 1) * QB],
                            kT[hp:hp + 64, k0:k0 + KB],
                            qT[hp:hp + 64, q0:q0 + QB],
                            start=True, stop=True)
                    PT = ptp.tile([128, 2 * QB], F32R, tag="PT")
                    nc.scalar.activation(out=PT, in_=S, func=EXP, scale=0.125)
                    for h in range(H_LOC):
                        nc.tensor.matmul(
                            OT[h],
                            Vp[:, kbg, 65 * h:65 * h + 65],
                            PT[:, h * QB:(h + 1) * QB],
                            start=(kb == 0), stop=(kb == NKB - 1))
                # epilogue: normalize by the softmax denominator (row 64 of
                # each OT). Drain psum fast; all the slow ops run decoupled.
                Osb = [onp.tile([64, QB], F32, tag=f"Os{h}", name=f"Os{h}")
                       for h in range(H_LOC)]
                dnb = smp.tile([1, 2 * QB], F32, tag="dn")
                for h in range(H_LOC):
                    nc.vector.tensor_copy(out=Osb[h], in_=OT[h][0:64, :])
                    nc.vector.tensor_copy(out=dnb[:, h * QB:(h + 1) * QB],
                                          in_=OT[h][64:65, :])
                nc.vector.reciprocal_approx_fast(out=rec_r[0:1, :], in_=dnb)
                for h in range(H_LOC):
                    Bc = psQ.tile([128, QB], F32, tag="pq", name=f"bc{h}")
                    nc.tensor.matmul(Bc, onesK,
                                     rec_r[:, h * QB:(h + 1) * QB],
                                     start=True, stop=True)
                    On = onp.tile([64, QB], F32, tag=f"On{h}")
                    nc.vector.tensor_mul(On, Osb[h], Bc[0:64, :])
                    # scalar-queue HWDGE: keeps output stores off the sync
                    # queue so they never delay a slab load mid-ramp
                    nc.scalar.dma_start(
                        out=out_ap[b, h, :, qj * QB:(qj + 1) * QB],
                        in_=On)


_CACHED_NC = None


def _build():
    global _CACHED_NC
    if _CACHED_NC is not None:
        return _CACHED_NC
    nc = bacc.Bacc("TRN2", target_bir_lowering=False, debug=False,
                   num_devices=NCORES)
    x = nc.dram_tensor("x", [NSLAB, 128, 8, SLAB], F32R,
                       kind="ExternalInput").ap()
    w = nc.dram_tensor("w", [DIM, E], F32R, kind="ExternalInput").ap()
    out = nc.dram_tensor("out", [B, H_LOC, HD, N], F32,
                         kind="ExternalOutput").ap()
    with tile.TileContext(nc) as tc:
        _emit(tc, out, x, w)
    nc.compile()
    _CACHED_NC = nc
    return nc


def _w_slice(w_qkv: np.ndarray, core: int) -> np.ndarray:
    cols = []
    for part in range(3):  # q, k, v column groups of w_qkv
        base = part * HEADS * HD + core * H_LOC * HD
        cols.append(w_qkv[:, base:base + H_LOC * HD])
    return np.ascontiguousarray(np.concatenate(cols, axis=1))


def kernel(x: np.ndarray, w_qkv: np.ndarray, _trace: bool = False):
    nc = _build()
    x = np.asarray(x, dtype=np.float32)
    w_qkv = np.asarray(w_qkv, dtype=np.float32)
    # pre-slabbed transpose: xup[s, p, c, j] = x[s*512+j, c*128+p]
    xup = np.ascontiguousarray(
        x.reshape(NSLAB, SLAB, 8, 128).transpose(0, 3, 2, 1))
    in_maps = [{"x": xup, "w": _w_slice(w_qkv, i)} for i in range(NCORES)]
    res = run_bass_kernel_spmd(nc, in_maps, list(range(NCORES)), trace=_trace)
    out = np.empty((B, HEADS, N, HD), np.float32)
    for i in range(NCORES):
        o = res.results[i]["out"]  # [B, H_LOC, HD, N]
        for hl in range(H_LOC):
            out[:, i * H_LOC + hl] = np.swapaxes(o[:, hl], 1, 2)
    if _trace:
        kernel.last_exec_time_ns = res.exec_time_ns
    return out


# revision 26
# speedup vs baseline: 1.8675x; 1.1783x over previous
"""Fused multi-head attention (QKV projection + softmax attention) on 8 TRN2
NeuronCores.

Problem: x [2, 2048, 1024] f32, w_qkv [1024, 3072] f32 ->
         out [2, 16, 2048, 64] f32   (16 heads, head_dim 64)

Sharding: tensor-parallel over heads. Each of the 8 cores owns 2 heads: it
gets the full x (pre-transposed AND pre-slabbed on host so every DMA line is
16 KB contiguous) plus its [1024, 384] slice of w_qkv columns and computes
its slice of the output independently. No collectives. Output leaves the
device as O^T per head [b, h, d, n]; the host transposes back to
[b, h, n, d] while unsharding.

Per-core pipeline (everything f32r on the PE = full-rate fp32):
  1. QKV^T [e, n] = w.T @ xT, streamed in 8 slabs of 512 n-columns; psQ is
     double-buffered so accumulation never waits on the DVE drain. Emission
     is split per batch (slabs 0-3, attention b0, slabs 4-7, attention b1)
     so batch-0 attention outranks batch-1 projection on the PE.
  2. V blocks flipped to natural [k, d] via one full 128x128 PE transpose
     per key-block, stored as Vp = [V_h0 | ones | V_h1 | ones] (65-column
     stationary per head: the ones column accumulates the softmax
     denominator inside the same PV matmul).
  3. Attention: a flat software-pipelined stream over (query-block,
     key-block) steps:
       S^T[k, q] = K^T.T @ Q^T    (2 heads row-tiled -> concurrent)
       P^T = exp(S^T/8)            (Scalar engine: ~1.15us per [128, 1024]
                                    block, 128 blocks = the critical path)
       O^T/denom (+= over kb) = [V_h | 1].T @ P^T_h  (M=65, serial pair)
     The NEXT step's score matmuls are emitted BEFORE this step's PV so the
     PE queue never stalls the exp stream behind an exp-dependent matmul
     (without this the ACT engine loses ~230 ns per step).
  4. Epilogue per query block, decoupled: psum drained immediately (DVE),
     one reciprocal_approx_fast into rows {0,32} of a zero-padded rhs, one
     K=64 fp32 matmul against an uploaded selector broadcasts both heads'
     1/denom across partitions; the exp-dependent matmul chain is emitted
     one step late (pending list) to keep it out of the hot PE window.
"""

import numpy as np

import concourse.bass as bass
import concourse.tile as tile
from concourse import bacc, mybir
from concourse.bass_utils import run_bass_kernel_spmd
from concourse.masks import make_identity

F32 = mybir.dt.float32
F32R = mybir.dt.float32r
EXP = mybir.ActivationFunctionType.Exp

B = 2
N = 2048                 # sequence length per batch
NT = B * N               # 4096 columns in xT / qkvT
DIM = 1024
HEADS = 16
HD = 64
NCORES = 8
H_LOC = HEADS // NCORES  # 2 heads per core
E = 3 * H_LOC * HD       # 384 w columns per core
NSLAB = 8
SLAB = NT // NSLAB       # 512 n-columns per projection slab
QB = 512                 # query block
NQB = N // QB            # 4 query blocks per batch
KB = 128                 # key block
NKB = N // KB            # 16 key blocks per batch


def _emit(tc, out_ap, x_ap, w_ap):
    nc = tc.nc
    from contextlib import ExitStack
    ctx = ExitStack()
    with ctx:
        const = ctx.enter_context(tc.tile_pool(name="const", bufs=1))
        xsp = ctx.enter_context(tc.tile_pool(name="xsp", bufs=3))
        qkv = ctx.enter_context(tc.tile_pool(name="qkv", bufs=1))
        vpp = ctx.enter_context(tc.tile_pool(name="vpp", bufs=1))
        ptp = ctx.enter_context(tc.tile_pool(name="ptp", bufs=4))
        smp = ctx.enter_context(tc.tile_pool(name="smp", bufs=4))
        onp = ctx.enter_context(tc.tile_pool(name="onp", bufs=2))

        psS = ctx.enter_context(tc.tile_pool(name="psS", bufs=2, space="PSUM"))
        psO = ctx.enter_context(tc.tile_pool(name="psO", bufs=2, space="PSUM"))
        psQ = ctx.enter_context(tc.tile_pool(name="psQ", bufs=2, space="PSUM"))

        # ---- constants ----
        ident = const.tile([128, 128], F32)
        make_identity(nc, ident)
        onesK = const.tile([64, 128], F32)   # denom-broadcast stationary
        nc.vector.memset(onesK, 1.0)
        # persistent rhs for the broadcast matmuls: row 0 carries the two
        # heads' 1/denom, rows 1..63 stay zero forever.
        # (reciprocal_approx_fast writes/reads are broken on HW at non-zero
        # base partitions -> everything recip stays at partition 0.)
        rec_r = qkv.tile([64, 2 * QB], F32, name="rec_r")
        nc.vector.memset(rec_r, 0.0)
        # preload the exp table set early (overlaps the w DMA)
        warm = const.tile([1, 1], F32)
        nc.scalar.activation(out=warm, in_=ident[0:1, 0:1], func=EXP)

        w_sb = const.tile([128, 8, E], F32R)
        nc.sync.dma_start(out=w_sb,
                          in_=w_ap.rearrange("(c p) e -> p c e", p=128))

        qT = qkv.tile([128, NT], F32R, name="qT")
        kT = qkv.tile([128, NT], F32R, name="kT")
        vT = qkv.tile([128, NT], F32, name="vT")
        # Vp[:, kb] = [V_h0 | ones | V_h1 | ones], natural [k, d] layout
        Vp = vpp.tile([128, B * NKB, 130], F32R, name="Vp")
        Vp_td = Vp.rearrange("p k (t u) -> p k t u", t=2)
        ones32 = const.tile([128, B * NKB], F32)
        nc.vector.memset(ones32, 1.0)
        nc.vector.tensor_copy(out=Vp[:, :, 64:65].squeeze(2), in_=ones32)
        nc.vector.tensor_copy(out=Vp[:, :, 129:130].squeeze(2), in_=ones32)

        def emit_proj(s):
            n0 = s * SLAB
            xs = xsp.tile([128, 8, SLAB], F32R, tag="xs", name="xs")
            nc.sync.dma_start(out=xs, in_=x_ap[s])
            for eb in range(3):
                tgt = (qT, kT, vT)[eb]
                ps = psQ.tile([128, SLAB], F32, tag="pq", name=f"pq{eb}")
                for dc in range(8):
                    nc.tensor.matmul(
                        ps,
                        w_sb[:, dc, eb * 128:(eb + 1) * 128],
                        xs[:, dc, :],
                        start=(dc == 0), stop=(dc == 7))
                nc.vector.tensor_copy(out=tgt[:, n0:n0 + SLAB], in_=ps)
            # V to natural [k, d] (one full 128x128 transpose per key-block)
            for j in range(4):
                kbg = s * 4 + j
                tp = psQ.tile([128, SLAB], F32, tag="pq", name="vt")
                nc.tensor.transpose(
                    tp[:, 0:128], vT[:, kbg * 128:(kbg + 1) * 128], ident)
                nc.vector.tensor_copy(
                    out=Vp_td[:, kbg, :, 0:64],
                    in_=tp[:, 0:128].rearrange("p (t u) -> p t u", t=2))

        def emit_scores(b, qj, kb):
            q0 = b * N + qj * QB
            k0 = b * N + kb * KB
            S = psS.tile([128, 2 * QB], F32, tag="S", name="S")
            for h in range(H_LOC):
                hp = 64 * h
                nc.tensor.matmul(
                    S[:, h * QB:(h + 1) * QB],
                    kT[hp:hp + 64, k0:k0 + KB],
                    qT[hp:hp + 64, q0:q0 + QB],
                    start=True, stop=True)
            return S

        pending = []

        def flush_pending():
            while pending:
                pending.pop(0)()

        def emit_attention(b):
            steps = [(qj, kb) for qj in range(NQB) for kb in range(NKB)]
            S_cur = emit_scores(b, 0, 0)
            OT = None
            for i, (qj, kb) in enumerate(steps):
                if kb == 0:
                    OT = [psO.tile([65, QB], F32, tag="OT", name=f"OT{h}")
                          for h in range(H_LOC)]
                PT = ptp.tile([128, 2 * QB], F32R, tag="PT", name="PT")
                nc.scalar.activation(out=PT, in_=S_cur, func=EXP, scale=0.125)
                if i + 1 < len(steps):
                    S_cur = emit_scores(b, *steps[i + 1])
                if kb == 1:
                    flush_pending()
                kbg = b * NKB + kb
                for h in range(H_LOC):
                    nc.tensor.matmul(
                        OT[h],
                        Vp[:, kbg, 65 * h:65 * h + 65],
                        PT[:, h * QB:(h + 1) * QB],
                        start=(kb == 0), stop=(kb == NKB - 1))
                if kb == NKB - 1:
                    emit_epilogue(b, qj, OT)
            flush_pending()

        def emit_epilogue(b, qj, OT):
            # immediate psum drain (DVE only)
            Osb = [onp.tile([64, QB], F32, tag=f"Os{h}", name=f"Os{h}")
                   for h in range(H_LOC)]
            dnb = smp.tile([1, 2 * QB], F32, tag="dn", name="dnb")
            for h in range(H_LOC):
                nc.vector.tensor_copy(out=Osb[h], in_=OT[h][0:64, :])
                nc.vector.tensor_copy(out=dnb[:, h * QB:(h + 1) * QB],
                                      in_=OT[h][64:65, :])

            def rest():
                nc.vector.reciprocal_approx_fast(out=rec_r[0:1, :], in_=dnb)
                for h in range(H_LOC):
                    Bc = psQ.tile([128, QB], F32, tag="pq", name=f"bc{h}")
                    nc.tensor.matmul(Bc, onesK,
                                     rec_r[:, h * QB:(h + 1) * QB],
                                     start=True, stop=True)
                    On = onp.tile([64, QB], F32, tag=f"On{h}", name=f"On{h}")
                    nc.vector.tensor_mul(On, Osb[h],
                                         Bc[h * 64:(h + 1) * 64, :])
                    # scalar-queue HWDGE keeps output stores off the sync
                    # queue (slab loads)
                    nc.scalar.dma_start(
                        out=out_ap[b, h, :, qj * QB:(qj + 1) * QB],
                        in_=On)

            pending.append(rest)

        # ---- emission: per-batch projection, then that batch's attention
        for s in range(4):
            emit_proj(s)
        emit_attention(0)
        for s in range(4, 8):
            emit_proj(s)
        emit_attention(1)


_CACHED_NC = None


def _build():
    global _CACHED_NC
    if _CACHED_NC is not None:
        return _CACHED_NC
    nc = bacc.Bacc("TRN2", target_bir_lowering=False, debug=False,
                   num_devices=NCORES)
    x = nc.dram_tensor("x", [NSLAB, 128, 8, SLAB], F32R,
                       kind="ExternalInput").ap()
    w = nc.dram_tensor("w", [DIM, E], F32R, kind="ExternalInput").ap()
    out = nc.dram_tensor("out", [B, H_LOC, HD, N], F32,
                         kind="ExternalOutput").ap()
    with tile.TileContext(nc) as tc:
        _emit(tc, out, x, w)
    nc.compile()
    _CACHED_NC = nc
    return nc


def _w_slice(w_qkv: np.ndarray, core: int) -> np.ndarray:
    cols = []
    for part in range(3):  # q, k, v column groups of w_qkv
        base = part * HEADS * HD + core * H_LOC * HD
        cols.append(w_qkv[:, base:base + H_LOC * HD])
    return np.ascontiguousarray(np.concatenate(cols, axis=1))


def kernel(x: np.ndarray, w_qkv: np.ndarray, _trace: bool = False):
    nc = _build()
    x = np.asarray(x, dtype=np.float32)
    w_qkv = np.asarray(w_qkv, dtype=np.float32)
    # pre-slabbed transpose: xup[s, p, c, j] = x[s*512+j, c*128+p]
    xup = np.ascontiguousarray(
        x.reshape(NSLAB, SLAB, 8, 128).transpose(0, 3, 2, 1))
    in_maps = [{"x": xup, "w": _w_slice(w_qkv, i)} for i in range(NCORES)]
    res = run_bass_kernel_spmd(nc, in_maps, list(range(NCORES)), trace=_trace)
    out = np.empty((B, HEADS, N, HD), np.float32)
    for i in range(NCORES):
        o = res.results[i]["out"]  # [B, H_LOC, HD, N]
        for hl in range(H_LOC):
            out[:, i * H_LOC + hl] = np.swapaxes(o[:, hl], 1, 2)
    if _trace:
        kernel.last_exec_time_ns = res.exec_time_ns
    return out


# revision 29
# speedup vs baseline: 1.9416x; 1.0397x over previous
"""Fused multi-head attention (QKV projection + softmax attention) on 8 TRN2
NeuronCores.

Problem: x [2, 2048, 1024] f32, w_qkv [1024, 3072] f32 ->
         out [2, 16, 2048, 64] f32   (16 heads, head_dim 64)

Sharding: tensor-parallel over heads. Each of the 8 cores owns 2 heads: it
gets the full x (pre-transposed AND pre-slabbed on host so every DMA line is
16 KB contiguous) plus its [1024, 384] slice of w_qkv columns and computes
its slice of the output independently. No collectives. Output leaves the
device as O^T per head [b, h, d, n]; the host transposes back to
[b, h, n, d] while unsharding.

Per-core pipeline (everything f32r on the PE = full-rate fp32):
  1. QKV^T [e, n] = w.T @ xT, streamed in 8 slabs of 512 n-columns; psQ is
     double-buffered so accumulation never waits on the DVE drain. Emission
     is split per batch (slabs 0-3, attention b0, slabs 4-7, attention b1)
     so batch-0 attention outranks batch-1 projection on the PE.
  2. V blocks flipped to natural [k, d] via one full 128x128 PE transpose
     per key-block, stored as Vp = [V_h0 | ones | V_h1 | ones] (65-column
     stationary per head: the ones column accumulates the softmax
     denominator inside the same PV matmul).
  3. Attention: a flat software-pipelined stream over (query-block,
     key-block) steps:
       S^T[k, q] = K^T.T @ Q^T    (2 heads row-tiled -> concurrent)
       P^T = exp(S^T/8)            (Scalar engine: ~1.15us per [128, 1024]
                                    block, 128 blocks = the critical path)
       O^T/denom (+= over kb) = [V_h | 1].T @ P^T_h  (M=65, serial pair)
     The NEXT step's score matmuls are emitted BEFORE this step's PV so the
     PE queue never stalls the exp stream behind an exp-dependent matmul
     (without this the ACT engine loses ~230 ns per step).
  4. Epilogue per query block, decoupled: psum drained immediately (DVE),
     one reciprocal_approx_fast into rows {0,32} of a zero-padded rhs, one
     K=64 fp32 matmul against an uploaded selector broadcasts both heads'
     1/denom across partitions; the exp-dependent matmul chain is emitted
     one step late (pending list) to keep it out of the hot PE window.
"""

import numpy as np

import concourse.bass as bass
import concourse.tile as tile
from concourse import bacc, mybir
from concourse.bass_utils import run_bass_kernel_spmd
from concourse.masks import make_identity

F32 = mybir.dt.float32
F32R = mybir.dt.float32r
EXP = mybir.ActivationFunctionType.Exp

B = 2
N = 2048                 # sequence length per batch
NT = B * N               # 4096 columns in xT / qkvT
DIM = 1024
HEADS = 16
HD = 64
NCORES = 8
H_LOC = HEADS // NCORES  # 2 heads per core
E = 3 * H_LOC * HD       # 384 w columns per core
NSLAB = 8
SLAB = NT // NSLAB       # 512 n-columns per projection slab
QB = 512                 # query block
NQB = N // QB            # 4 query blocks per batch
KB = 128                 # key block
NKB = N // KB            # 16 key blocks per batch


def _emit(tc, out_ap, x_ap, w_ap):
    nc = tc.nc
    from contextlib import ExitStack
    ctx = ExitStack()
    with ctx:
        const = ctx.enter_context(tc.tile_pool(name="const", bufs=1))
        xsp = ctx.enter_context(tc.tile_pool(name="xsp", bufs=3))
        qkv = ctx.enter_context(tc.tile_pool(name="qkv", bufs=1))
        vpp = ctx.enter_context(tc.tile_pool(name="vpp", bufs=1))
        ptp = ctx.enter_context(tc.tile_pool(name="ptp", bufs=4))
        smp = ctx.enter_context(tc.tile_pool(name="smp", bufs=4))
        onp = ctx.enter_context(tc.tile_pool(name="onp", bufs=2))

        psS = ctx.enter_context(tc.tile_pool(name="psS", bufs=2, space="PSUM"))
        psO = ctx.enter_context(tc.tile_pool(name="psO", bufs=2, space="PSUM"))
        psQ = ctx.enter_context(tc.tile_pool(name="psQ", bufs=2, space="PSUM"))

        # ---- constants ----
        ident = const.tile([128, 128], F32)
        make_identity(nc, ident)
        onesK = const.tile([64, 128], F32)   # denom-broadcast stationary
        nc.vector.memset(onesK, 1.0)
        # persistent rhs for the broadcast matmuls: row 0 carries the two
        # heads' 1/denom, rows 1..63 stay zero forever.
        # (reciprocal_approx_fast writes/reads are broken on HW at non-zero
        # base partitions -> everything recip stays at partition 0.)
        rec_r = qkv.tile([64, 2 * QB], F32, name="rec_r")
        nc.vector.memset(rec_r, 0.0)
        # preload the exp table set early (overlaps the w DMA)
        warm = const.tile([1, 1], F32)
        nc.scalar.activation(out=warm, in_=ident[0:1, 0:1], func=EXP)

        w_sb = const.tile([128, 8, E], F32R)
        nc.sync.dma_start(out=w_sb,
                          in_=w_ap.rearrange("(c p) e -> p c e", p=128))

        qT = qkv.tile([128, NT], F32R, name="qT")
        kT = qkv.tile([128, NT], F32R, name="kT")
        vT = qkv.tile([128, NT], F32, name="vT")
        # Vp[:, kb] = [V_h0 | ones | V_h1 | ones], natural [k, d] layout
        Vp = vpp.tile([128, B * NKB, 130], F32R, name="Vp")
        Vp_td = Vp.rearrange("p k (t u) -> p k t u", t=2)
        ones32 = const.tile([128, B * NKB], F32)
        nc.vector.memset(ones32, 1.0)
        nc.vector.tensor_copy(out=Vp[:, :, 64:65].squeeze(2), in_=ones32)
        nc.vector.tensor_copy(out=Vp[:, :, 129:130].squeeze(2), in_=ones32)

        def emit_proj(s):
            n0 = s * SLAB
            xs = xsp.tile([128, 8, SLAB], F32R, tag="xs", name="xs")
            nc.sync.dma_start(out=xs, in_=x_ap[s])
            for eb in range(3):
                tgt = (qT, kT, vT)[eb]
                ps = psQ.tile([128, SLAB], F32, tag="pq", name=f"pq{eb}")
                for dc in range(8):
                    nc.tensor.matmul(
                        ps,
                        w_sb[:, dc, eb * 128:(eb + 1) * 128],
                        xs[:, dc, :],
                        start=(dc == 0), stop=(dc == 7))
                nc.vector.tensor_copy(out=tgt[:, n0:n0 + SLAB], in_=ps)
            # V to natural [k, d] (one full 128x128 transpose per key-block)
            for j in range(4):
                kbg = s * 4 + j
                tp = psQ.tile([128, SLAB], F32, tag="pq", name="vt")
                nc.tensor.transpose(
                    tp[:, 0:128], vT[:, kbg * 128:(kbg + 1) * 128], ident)
                nc.vector.tensor_copy(
                    out=Vp_td[:, kbg, :, 0:64],
                    in_=tp[:, 0:128].rearrange("p (t u) -> p t u", t=2))

        def emit_scores(b, qj, kb):
            q0 = b * N + qj * QB
            k0 = b * N + kb * KB
            S = psS.tile([128, 2 * QB], F32, tag="S", name="S")
            for h in range(H_LOC):
                hp = 64 * h
                nc.tensor.matmul(
                    S[:, h * QB:(h + 1) * QB],
                    kT[hp:hp + 64, k0:k0 + KB],
                    qT[hp:hp + 64, q0:q0 + QB],
                    start=True, stop=True)
            return S

        pending = []

        def flush_pending():
            while pending:
                pending.pop(0)()

        def emit_attention(b, proj_hooks=None):
            steps = [(qj, kb) for qj in range(NQB) for kb in range(NKB)]
            S_cur = emit_scores(b, 0, 0)
            OT = None
            for i, (qj, kb) in enumerate(steps):
                if kb == 0:
                    OT = [psO.tile([65, QB], F32, tag="OT", name=f"OT{h}")
                          for h in range(H_LOC)]
                PT = ptp.tile([128, 2 * QB], F32R, tag="PT", name="PT")
                nc.scalar.activation(out=PT, in_=S_cur, func=EXP, scale=0.125)
                if i + 1 < len(steps):
                    S_cur = emit_scores(b, *steps[i + 1])
                if kb == 1:
                    flush_pending()
                    # interleave next batch's projection into this batch's
                    # ACT-bound stretch (the PE absorbs it in its slack)
                    if proj_hooks and qj in proj_hooks:
                        for s in proj_hooks[qj]:
                            emit_proj(s)
                kbg = b * NKB + kb
                for h in range(H_LOC):
                    nc.tensor.matmul(
                        OT[h],
                        Vp[:, kbg, 65 * h:65 * h + 65],
                        PT[:, h * QB:(h + 1) * QB],
                        start=(kb == 0), stop=(kb == NKB - 1))
                if kb == NKB - 1:
                    emit_epilogue(b, qj, OT)
            flush_pending()

        def emit_epilogue(b, qj, OT):
            # immediate psum drain (DVE only)
            Osb = [onp.tile([64, QB], F32, tag=f"Os{h}", name=f"Os{h}")
                   for h in range(H_LOC)]
            dnb = smp.tile([1, 2 * QB], F32, tag="dn", name="dnb")
            for h in range(H_LOC):
                nc.vector.tensor_copy(out=Osb[h], in_=OT[h][0:64, :])
                nc.vector.tensor_copy(out=dnb[:, h * QB:(h + 1) * QB],
                                      in_=OT[h][64:65, :])

            def rest():
                nc.vector.reciprocal_approx_fast(out=rec_r[0:1, :], in_=dnb)
                for h in range(H_LOC):
                    Bc = psQ.tile([128, QB], F32, tag="pq", name=f"bc{h}")
                    nc.tensor.matmul(Bc, onesK,
                                     rec_r[:, h * QB:(h + 1) * QB],
                                     start=True, stop=True)
                    On = onp.tile([64, QB], F32, tag=f"On{h}", name=f"On{h}")
                    nc.vector.tensor_mul(On, Osb[h],
                                         Bc[h * 64:(h + 1) * 64, :])
                    # gpsimd (SWDGE) queue: the gpsimd engine is otherwise
                    # idle, so the trigger's wait on the normalize chain
                    # can't block the exp stream (scalar queue) or slab
                    # loads (sync queue)
                    nc.gpsimd.dma_start(
                        out=out_ap[b, h, :, qj * QB:(qj + 1) * QB],
                        in_=On)

            pending.append(rest)

        # ---- emission: batch-0 projection, then batch-0 attention with
        # batch-1's projection interleaved into its ACT-bound stretch
        for s in range(4):
            emit_proj(s)
        emit_attention(0, proj_hooks={1: [4, 5], 2: [6, 7]})
        emit_attention(1)


_CACHED_NC = None


def _build():
    global _CACHED_NC
    if _CACHED_NC is not None:
        return _CACHED_NC
    nc = bacc.Bacc("TRN2", target_bir_lowering=False, debug=False,
                   num_devices=NCORES)
    x = nc.dram_tensor("x", [NSLAB, 128, 8, SLAB], F32R,
                       kind="ExternalInput").ap()
    w = nc.dram_tensor("w", [DIM, E], F32R, kind="ExternalInput").ap()
    out = nc.dram_tensor("out", [B, H_LOC, HD, N], F32,
                         kind="ExternalOutput").ap()
    with tile.TileContext(nc) as tc:
        _emit(tc, out, x, w)
    nc.compile()
    _CACHED_NC = nc
    return nc


def _w_slice(w_qkv: np.ndarray, core: int) -> np.ndarray:
    cols = []
    for part in range(3):  # q, k, v column groups of w_qkv
        base = part * HEADS * HD + core * H_LOC * HD
        cols.append(w_qkv[:, base:base + H_LOC * HD])
    return np.ascontiguousarray(np.concatenate(cols, axis=1))


def kernel(x: np.ndarray, w_qkv: np.ndarray, _trace: bool = False):
    nc = _build()
    x = np.asarray(x, dtype=np.float32)
    w_qkv = np.asarray(w_qkv, dtype=np.float32)
    # pre-slabbed transpose: xup[s, p, c, j] = x[s*512+j, c*128+p]
    xup = np.ascontiguousarray(
        x.reshape(NSLAB, SLAB, 8, 128).transpose(0, 3, 2, 1))
    in_maps = [{"x": xup, "w": _w_slice(w_qkv, i)} for i in range(NCORES)]
    res = run_bass_kernel_spmd(nc, in_maps, list(range(NCORES)), trace=_trace)
    out = np.empty((B, HEADS, N, HD), np.float32)
    for i in range(NCORES):
        o = res.results[i]["out"]  # [B, H_LOC, HD, N]
        for hl in range(H_LOC):
            out[:, i * H_LOC + hl] = np.swapaxes(o[:, hl], 1, 2)
    if _trace:
        kernel.last_exec_time_ns = res.exec_time_ns
    return out
